# revision 13
# baseline (speedup 1.0000x reference)
"""Causal single-head attention on 8 TRN2 NeuronCores.

Problem (hardcoded): x [4, 2048, 1024] f32; Wk, Wq, Wv [1024, 1024] f32.
  q = x @ Wk.T ; k = x @ Wq.T ; v = x @ Wv.T        (note ref's q/k weight swap)
  out = softmax(mask(q @ k.T) / sqrt(1024)) @ v

Sharding: 2 cores per batch, query-parallel.  Queries are carved into
256-query blocks; core h of a batch owns global blocks {0,3,4,7} (h=0) or
{1,2,5,6} (h=1).  Each core processes its four blocks as query tiles
T0..T3 with uniform key-context budgets of (4,8,12,16) 128-key chunks —
both cores' block contexts fit these budgets exactly, so block-causal
work is 40 chunk-slots per core instead of 48 for the 2x512-query split.
Every core runs the identical program (true SPMD); causality and padding
are encoded in per-core additive mask inputs.

Tiles are processed as adjacent PAIRS (T0|T1 and T2|T3, column-adjacent
in qt/out): while both tiles of a pair are active a single 512-wide
matmul serves both; once the short tile's budget is exhausted the pair's
PSUM keeps accumulating only the long tile's 256 columns at a column
offset.  This keeps baseline-sized instructions (and PSUM-bank usage)
while skipping the masked-out work.

K/V projections are fully pair-split: core h projects K and V only for
its own key half [1024h : 1024h+1024), then the halves are exchanged
with pair AllGathers through DRAM bounce buffers (K first — scores need
it earlier).  Both ranks read back BOTH gathered regions (region r holds
group-rank r's half), landing K/V in canonical key order on both ranks.
A tiny dependency-free warmup collective absorbs the CC stream's
first-op setup cost.  Readbacks are split across the Sync and GpSimd
queues (descriptor issue is ~600 ns, serialized per queue).

On-chip layout is feature-major (all host-side transposes are free):
  xT/wT in, Q^T/K^T feature-major, V sequence-major.  Scores are
  computed as S^T[k, q] so softmax needs no on-chip transpose: exp via
  ACT (no max subtraction — scaled scores are ~N(0,1)), sum-of-exp via a
  ones-column matmul, AV accumulates out^T[e, q] with V stationary.  The
  per-query 1/sum is broadcast across partitions with a K=1 PE matmul
  and applied by DVE during the PSUM->SBUF output copy.  Output returns
  as out^T, transposed back on the host.  All matmuls bf16 with fp32
  PSUM accumulation.
"""

import functools

import ml_dtypes
import numpy as np

B = 4
S = 2048
D = 1024
P = 128
DCH = D // P            # 8 contraction chunks
QT = 256                # query-tile width
HALF = S // 2           # own key half (pair-split projections)
NH = HALF // P          # 8 key slices per half
NKB = S // P            # 16 key chunks total
BUD = (4, 8, 12, 16)    # per-tile key-chunk budgets
NEG = np.float32(-30000.0)

# global 256-query block ids per (h, tile)
_QBLOCKS = ((0, 3, 4, 7), (1, 2, 5, 6))

_BF16 = ml_dtypes.bfloat16


@functools.lru_cache(maxsize=1)
def _build_nc():
    import concourse.bass as bass  # noqa: F401  (registers engines)
    import concourse.mybir as mybir
    from concourse import bacc, tile

    bf16 = mybir.dt.bfloat16
    f32 = mybir.dt.float32
    add = mybir.AluOpType.add
    mult = mybir.AluOpType.mult
    Exp = mybir.ActivationFunctionType.Exp
    PAIRS = [[2 * i, 2 * i + 1] for i in range(4)]

    nc = bacc.Bacc("TRN2", target_bir_lowering=False, debug=False, num_devices=8)

    xT = nc.declare_dram_parameter("xT", [D, HALF], bf16, isOutput=False)
    xqT = nc.declare_dram_parameter("xqT", [D, 4 * QT], bf16, isOutput=False)
    wqT = nc.declare_dram_parameter("wqT", [D, D], bf16, isOutput=False)
    wkT = nc.declare_dram_parameter("wkT", [D, D], bf16, isOutput=False)
    wvT = nc.declare_dram_parameter("wvT", [D, D], bf16, isOutput=False)
    # pair-wide mask chunks: rows 0:512 = pair01 k=0..3, rows 512:2048 =
    # pair23 k=0..11; single-tile chunks: rows 0:512 = T1 k=4..7,
    # rows 512:1024 = T3 k=12..15
    maskP = nc.declare_dram_parameter("maskP", [16 * P, 2 * QT], bf16,
                                      isOutput=False)
    maskS = nc.declare_dram_parameter("maskS", [8 * P, QT], bf16,
                                      isOutput=False)
    outT = nc.declare_dram_parameter("outT", [D, 4 * QT], f32, isOutput=True)

    with tile.TileContext(nc) as tc:
        with (
            tc.tile_pool(name="kv", bufs=1) as kv,
            tc.tile_pool(name="dram", bufs=1, space="DRAM") as dram,
        ):
            # ---- persistent SBUF tensors --------------------------------
            kt_sb = [kv.tile([P, S], bf16, tag=f"kt{e}", name=f"kt{e}")
                     for e in range(DCH)]
            qt_sb = [kv.tile([P, 4 * QT], bf16, tag=f"qt{e}", name=f"qt{e}")
                     for e in range(DCH)]
            v_sb = [kv.tile([P, D], bf16, tag=f"v{t}", name=f"v{t}")
                    for t in range(S // P)]
            mp_sb = [kv.tile([P, 2 * QT], bf16, tag=f"mp{k}", name=f"mp{k}")
                     for k in range(16)]
            ms_sb = [kv.tile([P, QT], bf16, tag=f"ms{k}", name=f"ms{k}")
                     for k in range(8)]
            ones_sb = kv.tile([P, 1], bf16, tag="ones", name="ones")
            nc.gpsimd.memset(ones_sb[:], 1.0)
            onesr = kv.tile([1, P], f32, tag="onesr", name="onesr")
            nc.gpsimd.memset(onesr[:], 1.0)
            # touch the Exp LUT once so the lazy activation-table load isn't
            # on the first score tile's critical path
            scr = kv.tile([P, 1], f32, tag="scr", name="scr")
            nc.scalar.activation(scr[:], ones_sb[:], Exp)

            # DRAM bounce buffers for the pair K/V exchange
            agin_k = dram.tile([D, HALF], bf16, name="agin_k")
            agout_k = dram.tile([2 * D, HALF], bf16, name="agout_k")
            # V is exchanged as two half-feature collectives so the first
            # can launch ~14 us earlier (the ~40 us launch-to-data latency
            # of each collective pipelines behind the K exchange)
            agin_va = dram.tile([HALF, D // 2], bf16, name="agin_va")
            agout_va = dram.tile([S, D // 2], bf16, name="agout_va")
            agin_vb = dram.tile([HALF, D // 2], bf16, name="agin_vb")
            agout_vb = dram.tile([S, D // 2], bf16, name="agout_vb")
            agin_w = dram.tile([2, 64], bf16, name="agin_w")
            agout_w = dram.tile([4, 64], bf16, name="agout_w")

            # ---- phase 1: load inputs + QKV projections -----------------
            with (
                tc.tile_pool(name="inp", bufs=1) as inp,
                tc.tile_pool(name="pps", bufs=2, space="PSUM") as pps,
            ):
                x_sb = [inp.tile([P, HALF], bf16, tag=f"x{d}", name=f"x{d}")
                        for d in range(DCH)]
                xq_sb = [inp.tile([P, 4 * QT], bf16, tag=f"xq{d}", name=f"xq{d}")
                         for d in range(DCH)]
                wq_sb = [inp.tile([P, D], bf16, tag=f"wq{d}", name=f"wq{d}")
                         for d in range(DCH)]
                wk_sb = [inp.tile([P, D], bf16, tag=f"wk{d}", name=f"wk{d}")
                         for d in range(DCH)]
                wv_sb = [inp.tile([P, D], bf16, tag=f"wv{d}", name=f"wv{d}")
                        for d in range(DCH)]
                # Whole-row loads in first-use order, spread across queues so
                # issue cost (~600 ns/descriptor, serialized per queue) does
                # not starve the K projection: x on Sync, wk on GpSimd, wv on
                # Scalar (ahead of the mask prefetch), wq/xq back on Sync.
                for d in range(DCH):
                    rows = slice(d * P, (d + 1) * P)
                    nc.sync.dma_start(out=x_sb[d][:], in_=xT[rows, :])
                    nc.gpsimd.dma_start(out=wk_sb[d][:], in_=wkT[rows, :])
                for d in range(DCH):
                    rows = slice(d * P, (d + 1) * P)
                    nc.scalar.dma_start(out=wv_sb[d][:], in_=wvT[rows, :])
                for d in range(DCH):
                    rows = slice(d * P, (d + 1) * P)
                    nc.sync.dma_start(out=wq_sb[d][:], in_=wqT[rows, :])
                    nc.sync.dma_start(out=xq_sb[d][:], in_=xqT[rows, :])
                # mask prefetch rides the Scalar queue behind wv; masks are
                # not needed until the score loop so this is fully hidden
                for k in range(16):
                    nc.scalar.dma_start(out=mp_sb[k][:],
                                        in_=maskP[k * P:(k + 1) * P, :])
                for k in range(8):
                    nc.scalar.dma_start(out=ms_sb[k][:],
                                        in_=maskS[k * P:(k + 1) * P, :])
                # Dependency-free warmup collective: the first op on the CC
                # stream pays a large one-time setup cost; burn it on a
                # throwaway exchange so the real K exchange moves data
                # sooner.  (Queued on GpSimd after the wk loads — the CC
                # instruction occupies its queue ~15 us for ring setup.)
                nc.gpsimd.collective_compute(
                    "AllGather", mybir.AluOpType.bypass,
                    replica_groups=PAIRS,
                    ins=[agin_w[:]], outs=[agout_w[:]],
                )

                # K^T for the own key half, staged into kt cols [0:HALF) and
                # bounced to DRAM per e-chunk so the exchange starts ASAP.
                for e in range(DCH):
                    esl = slice(e * P, (e + 1) * P)
                    pss = [pps.tile([P, 2 * QT], f32, tag=f"pj{i}",
                                    name=f"kps{e}_{i}") for i in range(2)]
                    for d in range(DCH):
                        for i in range(2):
                            nc.tensor.matmul(
                                pss[i][:], wk_sb[d][:, esl],
                                x_sb[d][:, i * 512:(i + 1) * 512],
                                start=(d == 0), stop=(d == DCH - 1),
                            )
                    for i in range(2):
                        nc.vector.tensor_copy(
                            kt_sb[e][:, i * 512:(i + 1) * 512], pss[i][:])
                    nc.sync.dma_start(out=agin_k[esl, :],
                                      in_=kt_sb[e][:, 0:HALF])
                nc.gpsimd.collective_compute(
                    "AllGather", mybir.AluOpType.bypass,
                    replica_groups=PAIRS,
                    ins=[agin_k[:]], outs=[agout_k[:]],
                )

                # V for the own key half (overlaps the K exchange).  Feature
                # columns 0:512 for ALL key slices first, bounced and
                # exchanged immediately; columns 512:1024 follow.
                for half, (agin, cols) in enumerate(
                        ((agin_va, slice(0, 512)), (agin_vb, slice(512, D)))):
                    for t in range(NH):
                        tsl = slice(t * P, (t + 1) * P)
                        ps = pps.tile([P, 2 * QT], f32, tag=f"pj{half}",
                                      name=f"vps{half}_{t}")
                        for d in range(DCH):
                            nc.tensor.matmul(
                                ps[:], x_sb[d][:, tsl], wv_sb[d][:, cols],
                                start=(d == 0), stop=(d == DCH - 1),
                            )
                        nc.vector.tensor_copy(v_sb[t][:, cols], ps[:])
                        nc.sync.dma_start(out=agin[tsl, :],
                                          in_=v_sb[t][:, cols])
                    nc.gpsimd.collective_compute(
                        "AllGather", mybir.AluOpType.bypass,
                        replica_groups=PAIRS,
                        ins=[agin[:]],
                        outs=[(agout_va if half == 0 else agout_vb)[:]],
                    )

                # K readback: BOTH regions (region r = group-rank r's half),
                # so the final key order is canonical on both ranks.  Split
                # across Sync and GpSimd to halve serialized issue.
                for e in range(DCH):
                    esl = slice(e * P, (e + 1) * P)
                    nc.sync.dma_start(out=kt_sb[e][:, 0:HALF],
                                      in_=agout_k[esl, :])
                    nc.gpsimd.dma_start(out=kt_sb[e][:, HALF:S],
                                        in_=agout_k[D + e * P:D + (e + 1) * P, :])

                # Q^T[e, q]: one wq weight tile drives both query halves.
                # Runs while the V exchange is in flight.
                for e in range(DCH):
                    esl = slice(e * P, (e + 1) * P)
                    pss = [pps.tile([P, 2 * QT], f32, tag=f"pj{qh}",
                                    name=f"qps{e}_{qh}") for qh in range(2)]
                    for d in range(DCH):
                        for qh in range(2):
                            nc.tensor.matmul(
                                pss[qh][:], wq_sb[d][:, esl],
                                xq_sb[d][:, qh * 512:(qh + 1) * 512],
                                start=(d == 0), stop=(d == DCH - 1),
                            )
                    for qh in range(2):
                        nc.vector.tensor_copy(
                            qt_sb[e][:, qh * 512:(qh + 1) * 512], pss[qh][:])

                # V readback (canonical order; slots 0..7 WAR on the bounce)
                for t in range(S // P):
                    eng = nc.sync if t % 2 == 0 else nc.gpsimd
                    eng.dma_start(out=v_sb[t][:, 0:512],
                                  in_=agout_va[t * P:(t + 1) * P, :])
                for t in range(S // P):
                    eng = nc.sync if t % 2 == 0 else nc.gpsimd
                    eng.dma_start(out=v_sb[t][:, 512:D],
                                  in_=agout_vb[t * P:(t + 1) * P, :])

            # ---- phase 2: attention over paired query tiles -------------
            with (
                tc.tile_pool(name="pp", bufs=1) as pp,
                tc.tile_pool(name="ost", bufs=4) as ost,
                tc.tile_pool(name="msc", bufs=2) as msc,
                tc.tile_pool(name="scp", bufs=4, space="PSUM") as scp,
                tc.tile_pool(name="aps", bufs=2, space="PSUM") as aps,
            ):
                # qt/out column ranges: T0 0:256, T1 256:512, T2 512:768,
                # T3 768:1024.  Pair01 spans 0:512, pair23 spans 512:1024.
                ph01, ph1, ph23, ph3 = [], [], [], []

                def exp_tile(store, prefix, w, ps):
                    ph = pp.tile([P, w], bf16, tag=f"{prefix}{len(store)}",
                                 name=f"ph_{prefix}{len(store)}")
                    nc.scalar.activation(ph[:], ps[:], Exp, scale=0.03125)
                    store.append(ph)

                # PSUM budget is exactly 8 banks: scp "sc" x4, aps "sum" x2
                # (sums + bc share the tag: each srow copy reads its sum slot
                # before the bc matmul recycles it), aps "av" x2.
                sum01 = aps.tile([1, 2 * QT], f32, tag="sum", name="sum01")
                sum23 = aps.tile([1, 2 * QT], f32, tag="sum", name="sum23")
                recb = {}

                def emit_sums01():
                    for kk in range(4):
                        nc.tensor.matmul(sum01[:, 0:2 * QT], ones_sb[:, 0:1],
                                         ph01[kk][:], start=(kk == 0),
                                         stop=False, skip_group_check=True)
                    for kk in range(4):
                        nc.tensor.matmul(sum01[:, QT:2 * QT], ones_sb[:, 0:1],
                                         ph1[kk][:], start=False,
                                         stop=(kk == 3), skip_group_check=True)

                def emit_sums23():
                    for kk in range(12):
                        nc.tensor.matmul(sum23[:, 0:2 * QT], ones_sb[:, 0:1],
                                         ph23[kk][:], start=(kk == 0),
                                         stop=False, skip_group_check=True)
                    for kk in range(4):
                        nc.tensor.matmul(sum23[:, QT:2 * QT], ones_sb[:, 0:1],
                                         ph3[kk][:], start=False,
                                         stop=(kk == 3), skip_group_check=True)

                def emit_chain(tag, sps):
                    srow = msc.tile([1, 2 * QT], f32, tag="srow",
                                    name=f"srow{tag}")
                    nc.vector.tensor_copy(srow[:], sps[:])
                    bc = aps.tile([P, 2 * QT], f32, tag="sum", name=f"bc{tag}")
                    nc.tensor.matmul(bc[:], onesr[:, 0:P], srow[:],
                                     start=True, stop=True)
                    rb = msc.tile([P, 2 * QT], f32, tag="recb",
                                  name=f"recb{tag}")
                    nc.vector.reciprocal_approx_fast(out=rb[:], in_=bc[:])
                    recb[tag] = rb

                # Score loop over key chunks.  Pair01 is live for k<4 plus
                # T1-only 256-wide tails for k in [4,8); pair23 live for
                # k<12 plus T3-only tails.  One kt slice load serves every
                # live tile.  Softmax sums are deferred one chunk so their
                # exp inputs are never on the PE critical path.
                for k in range(NKB):
                    ksl = slice(k * P, (k + 1) * P)
                    p01 = p1 = p23 = p3 = None
                    if k < 4:
                        p01 = scp.tile([P, 2 * QT], f32, tag="sc",
                                       name=f"sc01_{k}")
                    elif k < 8:
                        p1 = scp.tile([P, QT], f32, tag="sc", name=f"sc1_{k}")
                    if k < 12:
                        p23 = scp.tile([P, 2 * QT], f32, tag="sc",
                                       name=f"sc23_{k}")
                    else:
                        p3 = scp.tile([P, QT], f32, tag="sc", name=f"sc3_{k}")
                    for e in range(DCH):
                        kte = kt_sb[e]
                        if p01 is not None:
                            nc.tensor.matmul(
                                p01[:], kte[:, ksl], qt_sb[e][:, 0:512],
                                start=(e == 0), stop=(e == DCH - 1),
                                skip_group_check=True)
                        if p1 is not None:
                            nc.tensor.matmul(
                                p1[:], kte[:, ksl], qt_sb[e][:, QT:512],
                                start=(e == 0), stop=(e == DCH - 1),
                                skip_group_check=True)
                        if p23 is not None:
                            nc.tensor.matmul(
                                p23[:], kte[:, ksl], qt_sb[e][:, 512:1024],
                                start=(e == 0), stop=(e == DCH - 1),
                                skip_group_check=True)
                        if p3 is not None:
                            nc.tensor.matmul(
                                p3[:], kte[:, ksl], qt_sb[e][:, 768:1024],
                                start=(e == 0), stop=(e == DCH - 1),
                                skip_group_check=True)
                    # deferred softmax sums (inputs exp'd a full chunk ago)
                    if k == 8:
                        emit_sums01()
                    if k == 9:
                        emit_chain("01", sum01)
                    # mask + exp for this chunk's live tiles
                    if p01 is not None:
                        nc.vector.tensor_tensor(p01[:], p01[:], mp_sb[k][:],
                                                op=add)
                        exp_tile(ph01, "a", 2 * QT, p01)
                    if p1 is not None:
                        nc.vector.tensor_tensor(p1[:], p1[:],
                                                ms_sb[k - 4][:], op=add)
                        exp_tile(ph1, "b", QT, p1)
                    if p23 is not None:
                        nc.vector.tensor_tensor(p23[:], p23[:],
                                                mp_sb[4 + k][:], op=add)
                        exp_tile(ph23, "c", 2 * QT, p23)
                    if p3 is not None:
                        nc.vector.tensor_tensor(p3[:], p3[:],
                                                ms_sb[k - 8][:], op=add)
                        exp_tile(ph3, "d", QT, p3)

                emit_sums23()

                # AV.  Pair01 first (needs only V slices 0..7, which land
                # first from the exchange readback), then pair23.  The
                # pair's PSUM accumulates 512 wide while both tiles are
                # live, then 256 wide at a column offset for the long tile.
                def av_pair(pair, phw, pht, koff, tag):
                    csl = slice(0, 512) if pair == 0 else slice(512, 1024)
                    for e in range(DCH):
                        esl = slice(e * P, (e + 1) * P)
                        ps = aps.tile([P, 2 * QT], f32, tag="av",
                                      name=f"av{pair}_{e}")
                        nw = len(phw)
                        for kk in range(nw):
                            nc.tensor.matmul(
                                ps[:, 0:2 * QT], v_sb[kk][:, esl], phw[kk][:],
                                start=(kk == 0), stop=False,
                                skip_group_check=True)
                        for kk in range(4):
                            nc.tensor.matmul(
                                ps[:, QT:2 * QT], v_sb[koff + kk][:, esl],
                                pht[kk][:], start=False, stop=(kk == 3),
                                skip_group_check=True)
                        if pair == 0 and e == 0:
                            # tile-23 recip chain: its bc matmul rides here
                            # so the DVE sum-row copy has already landed
                            emit_chain("23", sum23)
                        ot = ost.tile([P, 2 * QT], f32, tag="ot",
                                      name=f"ot{pair}_{e}")
                        nc.vector.tensor_tensor(ot[:], ps[:], recb[tag][:],
                                                op=mult)
                        eng = nc.sync if e % 2 == 0 else nc.gpsimd
                        eng.dma_start(out=outT[esl, csl], in_=ot[:])

                av_pair(0, ph01, ph1, 4, "01")
                av_pair(1, ph23, ph3, 12, "23")

    nc.compile()
    return nc


def _make_mask(q0: int, k0: int, nk: int, w: int) -> np.ndarray:
    k = k0 + np.arange(nk * P)[:, None]
    q = q0 + np.arange(w)[None, :]
    return np.where(k <= q, np.float32(0.0), NEG).astype(_BF16)


def _build_masks(h: int):
    g = _QBLOCKS[h]
    q0 = [QT * gi for gi in g]
    # pair-wide chunks: pair01 k=0..3 then pair23 k=0..11
    mp = []
    for k in range(4):
        mp.append(np.concatenate(
            [_make_mask(q0[0], k * P, 1, QT), _make_mask(q0[1], k * P, 1, QT)],
            axis=1))
    for k in range(12):
        mp.append(np.concatenate(
            [_make_mask(q0[2], k * P, 1, QT), _make_mask(q0[3], k * P, 1, QT)],
            axis=1))
    # single-tile chunks: T1 k=4..7 then T3 k=12..15
    ms = [_make_mask(q0[1], k * P, 1, QT) for k in range(4, 8)]
    ms += [_make_mask(q0[3], k * P, 1, QT) for k in range(12, 16)]
    return (np.ascontiguousarray(np.concatenate(mp, axis=0)),
            np.ascontiguousarray(np.concatenate(ms, axis=0)))


def _in_maps(x, Wk, Wq, Wv):
    wq_t = np.ascontiguousarray(Wk.T.astype(_BF16))   # ref swap: q uses Wk
    wk_t = np.ascontiguousarray(Wq.T.astype(_BF16))
    wv_t = np.ascontiguousarray(Wv.T.astype(_BF16))
    masks = [_build_masks(0), _build_masks(1)]
    maps = []
    for c in range(8):
        b, h = divmod(c, 2)
        xb = x[b].astype(_BF16)
        # own key half only: this core projects K/V for keys
        # [HALF*h : HALF*(h+1)); the other half arrives via the exchange
        x_t = np.ascontiguousarray(xb[h * HALF:(h + 1) * HALF].T)
        xq_t = np.ascontiguousarray(np.concatenate(
            [xb[QT * g:QT * (g + 1)] for g in _QBLOCKS[h]], axis=0).T)
        maps.append({
            "xT": x_t,
            "xqT": xq_t,
            "wqT": wq_t,
            "wkT": wk_t,
            "wvT": wv_t,
            "maskP": masks[h][0],
            "maskS": masks[h][1],
        })
    return maps


def _assemble(results):
    out = np.empty((B, S, D), dtype=np.float32)
    for c, res in enumerate(results):
        b, h = divmod(c, 2)
        o = res["outT"]
        for i, g in enumerate(_QBLOCKS[h]):
            out[b, QT * g:QT * (g + 1)] = o[:, QT * i:QT * (i + 1)].T
    return out


def kernel(x, Wk, Wq, Wv, _trace=False):
    from concourse.bass_utils import run_bass_kernel_spmd

    nc = _build_nc()
    res = run_bass_kernel_spmd(nc, _in_maps(x, Wk, Wq, Wv), list(range(8)),
                               trace=_trace)
    out = _assemble(res.results)
    if _trace:
        return out, res
    return out
